# revision 34
# baseline (speedup 1.0000x reference)
"""Trainium2 Bass kernel for nn_MDCN (mixture-density head forward pass).

Reference computation (B=2048, F=1024, M=128):
    rho = tanh(feature @ h2rho_w.T + h2rho_b);  rho[:, 0] = 0.95
    pi  = softmax(feature @ h2pi_w.T + h2pi_b)
    var0 = exp(feature @ h2var_w.T + h2var_b)
    var = (1 - exp(rho)) * var0 + 1e-4
    W_ = r*muW + s*(r*(zstd/wstd)*(W-muW) + Z*s),  s = sqrt(1-r^2)
    mu = einsum('bmf,bf->bm', W_, feature)

Algebraic collapse (as before): with a = (zstd/wstd)*(W-muW),
    mu[b,m] = r*d1[b] + r*s*d2[b] + s^2*d3[b]
  where d1 = feature@muW, d2 = feature@a, d3 = feature@Z ride as 3 extra
matmul columns. s = sqrt(1-r^2) = (1+tanh u) * exp(-u) needs only Tanh+Exp.
The rho[:,0]=0.95 clamp is folded into the weights: rho weight column 0 is
zeroed and its bias set to atanh(0.95), so column 0 computes the constant
with no epilogue special-casing.

Pipeline structure (the original version was fully serial: 6.4us DMA +
4.7us matmul + 4.6us epilogue + 3.6us output = 20us; measured facts on
this part: per-core DMA tops out at ~220 GB/s no matter how many queues
or descriptor sizes, and the PE effectively runs at the 1.2 GHz mid
p-state for the whole iteration — warmup fillers never unthrottle it and
only delay the real matmuls):
  - Inputs stream on one HWDGE queue in arrival order = use order: rho+d
    weights, tile-0 features (so rho matmuls start ~3.5us in), tile-1
    features, then pi and var weight column groups.
  - PE chases the stream; each head's epilogue runs as soon as its
    group's psum closes, hiding under the remaining weight stream; only
    the var tail (exp + 2 DVE ops) trails the stream.
  - Epilogue ops are stacked across the two 128-row batch tiles via
    strided PSUM access patterns (psum tiles are [128, 2, 512] = 2
    banks), halving instruction-fixed costs. fp16 intermediates double
    DVE throughput. d1..d3 scalars are read directly out of PSUM.
  - Three fp16 output DMAs (one per head, partition-major layout for
    512B-contiguous descriptors), each dispatched as soon as its head
    finishes, spread across the SP/Act/SWDGE queues so their ~1.7us
    DGE startup latencies overlap; host upcasts to fp32.

Sharding: pure data-parallel over batch across 8 cores (256 rows/core),
weights replicated. No collectives (forward only).
"""

from contextlib import ExitStack

import numpy as np

import concourse.bass as bass
import concourse.bacc as bacc
import concourse.mybir as mybir
import concourse.tile as tile
from concourse.bass_utils import run_bass_kernel_spmd

B, F, M = 2048, 1024, 128
NCORES = 8
BC = B // NCORES            # 256 batch rows per core
NT = BC // 128              # 2 partition tiles per core
KC = F // 128               # 8 contraction chunks
GW_RHOD = M + 4             # rho group: 128 rho cols + d1,d2,d3 + pad
RHO_1 = np.float32(0.95)
TAU_INV = 1.0e-4
U0 = float(np.arctanh(np.float32(0.95)))   # folded rho[:,0] clamp

F32 = mybir.dt.float32
F16 = mybir.dt.float16
AF = mybir.ActivationFunctionType
OP = mybir.AluOpType

MM_NP = np.float16
# Measured: pre-warming the PE with filler matmuls does NOT unthrottle the
# clock-gate in time and the fillers just delay the real matmuls (in-order
# PE). Instead the stream is ordered so real matmuls start early (rho
# weights + tile-0 features first) and the PE busy-streak stays unbroken.
N_FILL = 0
N_FILL_MID = 0              # measured: PE stays at 1.2GHz; fillers only hurt


def _declare_io(nc):
    # ft[p, t, c, b] = feature[t*128+b, c*128+p]; stationary operand,
    # tile-major so each tile's half streams as one contiguous DMA.
    ft_dram = nc.dram_tensor("ft", [128, NT, KC, 128], F16,
                             kind="ExternalInput").ap()
    wpi_dram = nc.dram_tensor("wpi", [128, KC, M], F16,
                              kind="ExternalInput").ap()
    wrhod_dram = nc.dram_tensor("wrhod", [128, KC, GW_RHOD], F16,
                                kind="ExternalInput").ap()
    wvar_dram = nc.dram_tensor("wvar", [128, KC, M], F16,
                               kind="ExternalInput").ap()
    # blk: [ones(128) | bias_pi(128) | bias_rhod(132) | bias_var(128)]
    blk_dram = nc.dram_tensor("bias_blk", [1, 128 + M + GW_RHOD + M], F16,
                              kind="ExternalInput").ap()
    # Partition-major outputs (one per head): 512B-contiguous descriptors,
    # and each can ship as soon as its head finishes.
    opi_dram = nc.dram_tensor("out_pi", [128, NT * M], F16,
                              kind="ExternalOutput").ap()
    omu_dram = nc.dram_tensor("out_mu", [128, NT * M], F16,
                              kind="ExternalOutput").ap()
    ovar_dram = nc.dram_tensor("out_var", [128, NT * M], F16,
                               kind="ExternalOutput").ap()
    return (ft_dram, wpi_dram, wrhod_dram, wvar_dram, blk_dram,
            opi_dram, omu_dram, ovar_dram)


def _warmup_act(nc, consts):
    # Trigger the ACT exp/tanh table load immediately (costs ~2.7us once).
    warm_in = consts.tile([128, 1], F32, tag="warm_in", name="warm_in")
    warm_out = consts.tile([128, 1], F32, tag="warm_out", name="warm_out")
    nc.vector.memset(warm_in[:], 0.0)
    nc.scalar.activation(warm_out[:], warm_in[:], AF.Exp)


def _warmup_pe(nc, consts, scratch, n=None):
    # Filler matmuls: used only to bridge short PE idle gaps so the
    # busy-streak (and its p-state ramp) is not reset.
    wsrc = consts.tile([1, 128], F16, tag="pe_w", name="pe_w")
    nc.vector.memset(wsrc[:], 1.0)
    msrc = consts.tile([1, 512], F16, tag="pe_m", name="pe_m")
    nc.vector.memset(msrc[:], 1.0)
    for _ in range(N_FILL if n is None else n):
        nc.tensor.matmul(scratch[:], wsrc[:], msrc[:], start=True, stop=True)


def _emit_body(nc, tc, pools, drams, parts=("dma", "mm", "epi", "out")):
    consts, fwpool, psum, work = pools
    (ft_dram, wpi_dram, wrhod_dram, wvar_dram, blk_dram,
     opi_dram, omu_dram, ovar_dram) = drams

    BW = 128 + M + GW_RHOD + M
    blk = consts.tile([1, BW], F16, tag="bias_blk", name="bias_blk")
    nc.gpsimd.dma_start(blk[:], blk_dram)

    # Input stream, arrival order = use order: rho weights first, then
    # tile-0 features (tile-0 rho matmuls start ~2.5us in and keep the PE
    # busy-streak alive), tile-1 features, then pi and var weight groups.
    # Each group's epilogue chain hides under the remaining stream; only
    # var's short chain trails it.
    wrhod = fwpool.tile([128, KC, GW_RHOD], F16, tag="wrhod", name="wrhod")
    nc.sync.dma_start(wrhod[:], wrhod_dram)
    ft = fwpool.tile([128, NT, KC, 128], F16, tag="ft", name="ft")
    for t in range(NT):
        nc.sync.dma_start(ft[:, t], ft_dram[:, t])
    wpi = fwpool.tile([128, KC, M], F16, tag="wpi", name="wpi")
    nc.sync.dma_start(wpi[:], wpi_dram)
    wvar = fwpool.tile([128, KC, M], F16, tag="wvar", name="wvar")
    nc.sync.dma_start(wvar[:], wvar_dram)
    if "mm" not in parts:
        return

    # PSUM: one [128, 2, 512] tile (2 banks) per column group; matmul
    # dests are the per-tile halves, epilogue reads stacked strided APs.
    P_pi = psum.tile([128, NT, 512], F32, tag="P_pi", name="P_pi")
    P_rhod = psum.tile([128, NT, 512], F32, tag="P_rhod", name="P_rhod")
    P_var = psum.tile([128, NT, 512], F32, tag="P_var", name="P_var")
    scratch = psum.tile([128, 512], F32, tag="pe_scratch2", name="pe_scratch2")
    fw = consts.tile([1, 128], F16, tag="fill_w", name="fill_w")
    nc.vector.memset(fw[:], 1.0)
    fm = consts.tile([1, 512], F16, tag="fill_m", name="fill_m")
    nc.vector.memset(fm[:], 1.0)

    b_pi = blk[:, 128:128 + M]
    b_rhod = blk[:, 128 + M:128 + M + GW_RHOD]
    b_var = blk[:, 128 + M + GW_RHOD:BW]
    ones = blk[:, 0:128]
    for t in range(NT):
        nc.tensor.matmul(P_rhod[:, t, 0:GW_RHOD], ones, b_rhod,
                         start=True, stop=False)
        nc.tensor.matmul(P_pi[:, t, 0:M], ones, b_pi, start=True, stop=False)
        nc.tensor.matmul(P_var[:, t, 0:M], ones, b_var, start=True, stop=False)
    emit_epi = "epi" in parts
    emit_out = "out" in parts and emit_epi
    if emit_epi:
        # Flat [128, NT*M] output tiles -> one 512B descriptor/partition.
        # (o_var is declared 3D so the tail var op can be stacked across
        # both tiles in ONE DVE instruction; memory layout is identical.)
        o_pi = work.tile([128, NT * M], F16, tag="o_pi", name="o_pi")
        o_mu = work.tile([128, NT * M], F16, tag="o_mu", name="o_mu")
        o_var = work.tile([128, NT, M], F16, tag="o_var", name="o_var")

    # Each head's epilogue is EMITTED right after its matmul group: the
    # tile scheduler follows emission order per engine, so this biases it
    # to start each ACT/DVE chain at the earliest legal point instead of
    # batching them after later matmul groups.

    # --- rho group: matmuls per tile as its features land, then r/s/mu ---
    for c in range(KC):
        nc.tensor.matmul(P_rhod[:, 0, 0:GW_RHOD], ft[:, 0, c, :],
                         wrhod[:, c, :], start=False, stop=(c == KC - 1))
    for _ in range(N_FILL_MID):
        nc.tensor.matmul(scratch[:], fw[:], fm[:], start=True, stop=True)
    for c in range(KC):
        nc.tensor.matmul(P_rhod[:, 1, 0:GW_RHOD], ft[:, 1, c, :],
                         wrhod[:, c, :], start=False, stop=(c == KC - 1))
    if emit_epi:
        r = work.tile([128, NT, M], F16, tag="r", name="r")
        nc.scalar.activation(r[:], P_rhod[:, :, 0:M], AF.Tanh, scale=-1.0)
        eneg = work.tile([128, NT, M], F16, tag="eneg", name="eneg")
        nc.scalar.activation(eneg[:], P_rhod[:, :, 0:M], AF.Exp)
        erho = work.tile([128, NT, M], F16, tag="erho", name="erho")
        nc.scalar.activation(erho[:], r[:], AF.Exp)

        s = work.tile([128, NT, M], F16, tag="s", name="s")
        nc.vector.scalar_tensor_tensor(s[:], r[:], 1.0, eneg[:], OP.add,
                                       OP.mult)
        ss = work.tile([128, NT, M], F16, tag="ss", name="ss")
        nc.gpsimd.tensor_mul(ss[:], s[:], s[:])
        q = work.tile([128, NT, M], F16, tag="q", name="q")
        for t in range(NT):
            # q = d1 + s*d2; d-scalars read straight out of PSUM
            nc.vector.tensor_scalar(q[:, t, :], s[:, t, :],
                                    P_rhod[:, t, M + 1:M + 2],
                                    P_rhod[:, t, M:M + 1], OP.mult, OP.add)
        rq = work.tile([128, NT, M], F16, tag="rq", name="rq")
        nc.vector.tensor_mul(rq[:], r[:], q[:])
        for t in range(NT):
            # mu = ss*d3 + rq
            nc.vector.scalar_tensor_tensor(o_mu[:, t * M:(t + 1) * M],
                                           ss[:, t, :],
                                           P_rhod[:, t, M + 2:M + 3],
                                           rq[:, t, :], OP.mult, OP.add)
        if emit_out:
            nc.gpsimd.dma_start(omu_dram, o_mu[:])

    # --- pi group: matmuls, then softmax (stacked exp, DVE reduce) ---
    for c in range(KC):
        for t in range(NT):
            nc.tensor.matmul(P_pi[:, t, 0:M], ft[:, t, c, :], wpi[:, c, :],
                             start=False, stop=(c == KC - 1))
    if emit_epi:
        e_pi = work.tile([128, NT, M], F16, tag="e_pi", name="e_pi")
        nc.scalar.activation(e_pi[:], P_pi[:, :, 0:M], AF.Exp)
        ssum = work.tile([128, NT], F32, tag="ssum", name="ssum")
        nc.vector.tensor_reduce(ssum[:], e_pi[:], mybir.AxisListType.X,
                                OP.add)
        rsum = work.tile([128, NT], F32, tag="rsum", name="rsum")
        nc.vector.reciprocal(rsum[:], ssum[:])
        for t in range(NT):
            nc.vector.tensor_scalar_mul(o_pi[:, t * M:(t + 1) * M],
                                        e_pi[:, t, :], rsum[:, t:t + 1])
        if emit_out:
            nc.sync.dma_start(opi_dram, o_pi[:])

    # --- var group (tail): matmuls, then var = -(erho-1)*var0 + tau ---
    for c in range(KC):
        for t in range(NT):
            nc.tensor.matmul(P_var[:, t, 0:M], ft[:, t, c, :], wvar[:, c, :],
                             start=False, stop=(c == KC - 1))
    if emit_epi:
        ev = work.tile([128, NT, M], F16, tag="ev", name="ev")
        nc.scalar.activation(ev[:], P_var[:, :, 0:M], AF.Exp)
        t1 = work.tile([128, NT, M], F16, tag="t1", name="t1")
        nc.vector.scalar_tensor_tensor(t1[:], erho[:], 1.0, ev[:],
                                       OP.subtract, OP.mult)
        nc.vector.tensor_scalar(o_var[:], t1[:], -1.0, TAU_INV,
                                OP.mult, OP.add)
        if emit_out:
            nc.scalar.dma_start(ovar_dram.rearrange("p (t j) -> p t j", t=NT),
                                o_var[:])


def _build_pools(tc, ctx):
    consts = ctx.enter_context(tc.tile_pool(name="consts", bufs=1))
    fwpool = ctx.enter_context(tc.tile_pool(name="fw", bufs=1))
    psum = ctx.enter_context(tc.tile_pool(name="psum", bufs=1, space="PSUM"))
    work = ctx.enter_context(tc.tile_pool(name="work", bufs=1))
    return consts, fwpool, psum, work


def _build_nc():
    nc = bacc.Bacc("TRN2", target_bir_lowering=False, debug=False)
    drams = _declare_io(nc)
    with tile.TileContext(nc) as tc, ExitStack() as ctx:
        consts, fwpool, psum, work = _build_pools(tc, ctx)
        scratch = psum.tile([128, 512], F32, tag="pe_scratch",
                            name="pe_scratch")
        _warmup_act(nc, consts)
        _warmup_pe(nc, consts, scratch)
        _emit_body(nc, tc, (consts, fwpool, psum, work), drams)
    nc.compile()
    return nc


def build_loop_nc(reps, parts=("dma", "mm", "epi", "out"), fillers=True):
    """Timing variant: run the body `reps` times inside one NEFF (used only
    by the local test harness; the default full-barrier back-edge keeps
    iterations serialized so per-iter span ~ single-shot kernel time)."""
    nc = bacc.Bacc("TRN2", target_bir_lowering=False, debug=False)
    drams = _declare_io(nc)
    with tile.TileContext(nc) as tc, ExitStack() as ctx:
        consts, fwpool, psum, work = _build_pools(tc, ctx)
        scratch = psum.tile([128, 512], F32, tag="pe_scratch",
                            name="pe_scratch")
        _warmup_act(nc, consts)
        with tc.For_i(0, reps, 1):
            if fillers:
                _warmup_pe(nc, consts, scratch)
            _emit_body(nc, tc, (consts, fwpool, psum, work), drams,
                       parts=parts)
    nc.compile()
    return nc


_CACHE = {}


def _get_nc():
    if "nc" not in _CACHE:
        _CACHE["nc"] = _build_nc()
    return _CACHE["nc"]


def _host_prep(inputs):
    f32 = np.float32
    feature = np.ascontiguousarray(inputs["feature"], dtype=f32)
    muW = np.asarray(inputs["muW"], dtype=f32)
    W = np.asarray(inputs["W"], dtype=f32)
    Z = np.asarray(inputs["Z"], dtype=f32)
    logvarW = np.asarray(inputs["logvarW"], dtype=f32)
    logvarZ = np.asarray(inputs["logvarZ"], dtype=f32)

    wstd = np.sqrt(np.exp(logvarW)).astype(f32)
    zstd = np.sqrt(np.exp(logvarZ)).astype(f32)
    a = ((zstd / wstd).astype(f32) * (W - muW)).astype(f32)

    # Column groups; rho weights negated so psum = -u and exp(psum) = e^-u.
    wpi = np.asarray(inputs["h2pi_w"], dtype=f32).T          # [F, M]
    wrho = -np.asarray(inputs["h2rho_w"], dtype=f32).T       # [F, M]
    wrho[:, 0] = 0.0                                         # folded clamp
    wvar = np.asarray(inputs["h2var_w"], dtype=f32).T        # [F, M]
    wrhod = np.concatenate(
        [wrho, np.stack([muW, a, Z, np.zeros_like(muW)], axis=1)], axis=1)

    b_pi = np.asarray(inputs["h2pi_b"], dtype=f32)
    b_rho = -np.asarray(inputs["h2rho_b"], dtype=f32)
    b_rho[0] = -U0                                           # folded clamp
    b_var = np.asarray(inputs["h2var_b"], dtype=f32)
    blk = np.concatenate(
        [np.ones(128, dtype=f32), b_pi, b_rho, np.zeros(4, dtype=f32),
         b_var]).reshape(1, -1).astype(MM_NP)

    # [F, gw] -> [128(p), KC, gw]
    def wfmt(w):
        return np.ascontiguousarray(
            w.reshape(KC, 128, w.shape[1]).transpose(1, 0, 2), dtype=MM_NP)

    wpi_h, wrhod_h, wvar_h = wfmt(wpi), wfmt(wrhod), wfmt(wvar)

    in_maps = []
    for cr in range(NCORES):
        shard = feature[cr * BC:(cr + 1) * BC]               # [BC, F]
        # ft[p, t, c, b] = shard[t*128+b, c*128+p]
        ft = np.ascontiguousarray(
            shard.reshape(NT, 128, KC, 128).transpose(3, 0, 2, 1),
            dtype=MM_NP)
        in_maps.append({"ft": ft, "wpi": wpi_h, "wrhod": wrhod_h,
                        "wvar": wvar_h, "bias_blk": blk})
    return in_maps


def _postprocess(res, cores):
    outs = []
    for name in ("out_pi", "out_mu", "out_var"):
        # [128, NT*M] partition-major -> [BC, M] rows per core
        full = np.concatenate(
            [np.asarray(res.results[c][name], dtype=np.float32)
             .reshape(128, NT, M).transpose(1, 0, 2).reshape(BC, M)
             for c in cores], axis=0)
        outs.append(np.ascontiguousarray(full))
    return tuple(outs)


def kernel(**inputs):
    nc = _get_nc()
    in_maps = _host_prep(inputs)
    res = run_bass_kernel_spmd(nc, in_maps, list(range(NCORES)))
    return _postprocess(res, list(range(NCORES)))


# revision 35
# speedup vs baseline: 1.0186x; 1.0186x over previous
"""Trainium2 Bass kernel for nn_MDCN (mixture-density head forward pass).

Reference computation (B=2048, F=1024, M=128):
    rho = tanh(feature @ h2rho_w.T + h2rho_b);  rho[:, 0] = 0.95
    pi  = softmax(feature @ h2pi_w.T + h2pi_b)
    var0 = exp(feature @ h2var_w.T + h2var_b)
    var = (1 - exp(rho)) * var0 + 1e-4
    W_ = r*muW + s*(r*(zstd/wstd)*(W-muW) + Z*s),  s = sqrt(1-r^2)
    mu = einsum('bmf,bf->bm', W_, feature)

Algebraic collapse (as before): with a = (zstd/wstd)*(W-muW),
    mu[b,m] = r*d1[b] + r*s*d2[b] + s^2*d3[b]
  where d1 = feature@muW, d2 = feature@a, d3 = feature@Z ride as 3 extra
matmul columns. s = sqrt(1-r^2) = (1+tanh u) * exp(-u) needs only Tanh+Exp.
The rho[:,0]=0.95 clamp is folded into the weights: rho weight column 0 is
zeroed and its bias set to atanh(0.95), so column 0 computes the constant
with no epilogue special-casing.

Pipeline structure (the original version was fully serial: 6.4us DMA +
4.7us matmul + 4.6us epilogue + 3.6us output = 20us; measured facts on
this part: per-core DMA tops out at ~220 GB/s no matter how many queues
or descriptor sizes, and the PE effectively runs at the 1.2 GHz mid
p-state for the whole iteration — warmup fillers never unthrottle it and
only delay the real matmuls):
  - Inputs stream on one HWDGE queue in arrival order = use order: rho+d
    weights, tile-0 features (so rho matmuls start ~3.5us in), tile-1
    features, then pi and var weight column groups.
  - PE chases the stream; each head's epilogue runs as soon as its
    group's psum closes, hiding under the remaining weight stream; only
    the var tail (exp + 2 DVE ops) trails the stream.
  - Epilogue ops are stacked across the two 128-row batch tiles via
    strided PSUM access patterns (psum tiles are [128, 2, 512] = 2
    banks), halving instruction-fixed costs. fp16 intermediates double
    DVE throughput. d1..d3 scalars are read directly out of PSUM.
  - Three fp16 output DMAs (one per head, partition-major layout for
    512B-contiguous descriptors), each dispatched as soon as its head
    finishes, spread across the SP/Act/SWDGE queues so their ~1.7us
    DGE startup latencies overlap; host upcasts to fp32.

Sharding: pure data-parallel over batch across 8 cores (256 rows/core),
weights replicated. No collectives (forward only).
"""

from contextlib import ExitStack

import numpy as np

import concourse.bass as bass
import concourse.bacc as bacc
import concourse.mybir as mybir
import concourse.tile as tile
from concourse.bass_utils import run_bass_kernel_spmd

B, F, M = 2048, 1024, 128
NCORES = 8
BC = B // NCORES            # 256 batch rows per core
NT = BC // 128              # 2 partition tiles per core
KC = F // 128               # 8 contraction chunks
GW_RHOD = M + 4             # rho group: 128 rho cols + d1,d2,d3 + pad
RHO_1 = np.float32(0.95)
TAU_INV = 1.0e-4
U0 = float(np.arctanh(np.float32(0.95)))   # folded rho[:,0] clamp

F32 = mybir.dt.float32
F16 = mybir.dt.float16
AF = mybir.ActivationFunctionType
OP = mybir.AluOpType

MM_NP = np.float16
# Measured: pre-warming the PE with filler matmuls does NOT unthrottle the
# clock-gate in time and the fillers just delay the real matmuls (in-order
# PE). Instead the stream is ordered so real matmuls start early (rho
# weights + tile-0 features first) and the PE busy-streak stays unbroken.
N_FILL = 0
N_FILL_MID = 0              # measured: PE stays at 1.2GHz; fillers only hurt


def _declare_io(nc):
    # ft[p, t, c, b] = feature[t*128+b, c*128+p]; stationary operand,
    # tile-major so each tile's half streams as one contiguous DMA.
    ft_dram = nc.dram_tensor("ft", [128, NT, KC, 128], F16,
                             kind="ExternalInput").ap()
    wpi_dram = nc.dram_tensor("wpi", [128, KC, M], F16,
                              kind="ExternalInput").ap()
    wrhod_dram = nc.dram_tensor("wrhod", [128, KC, GW_RHOD], F16,
                                kind="ExternalInput").ap()
    wvar_dram = nc.dram_tensor("wvar", [128, KC, M], F16,
                               kind="ExternalInput").ap()
    # blk: [ones(128) | bias_pi(128) | bias_rhod(132) | bias_var(128)]
    blk_dram = nc.dram_tensor("bias_blk", [1, 128 + M + GW_RHOD + M], F16,
                              kind="ExternalInput").ap()
    # Partition-major outputs (one per head): 512B-contiguous descriptors,
    # and each can ship as soon as its head finishes.
    opi_dram = nc.dram_tensor("out_pi", [128, NT * M], F16,
                              kind="ExternalOutput").ap()
    omu_dram = nc.dram_tensor("out_mu", [128, NT * M], F16,
                              kind="ExternalOutput").ap()
    ovar_dram = nc.dram_tensor("out_var", [128, NT * M], F16,
                               kind="ExternalOutput").ap()
    return (ft_dram, wpi_dram, wrhod_dram, wvar_dram, blk_dram,
            opi_dram, omu_dram, ovar_dram)


def _warmup_act(nc, consts):
    # Trigger the ACT exp/tanh table load immediately (costs ~2.7us once).
    warm_in = consts.tile([128, 1], F32, tag="warm_in", name="warm_in")
    warm_out = consts.tile([128, 1], F32, tag="warm_out", name="warm_out")
    nc.vector.memset(warm_in[:], 0.0)
    nc.scalar.activation(warm_out[:], warm_in[:], AF.Exp)


def _warmup_pe(nc, consts, scratch, n=None):
    # Filler matmuls: used only to bridge short PE idle gaps so the
    # busy-streak (and its p-state ramp) is not reset.
    wsrc = consts.tile([1, 128], F16, tag="pe_w", name="pe_w")
    nc.vector.memset(wsrc[:], 1.0)
    msrc = consts.tile([1, 512], F16, tag="pe_m", name="pe_m")
    nc.vector.memset(msrc[:], 1.0)
    for _ in range(N_FILL if n is None else n):
        nc.tensor.matmul(scratch[:], wsrc[:], msrc[:], start=True, stop=True)


def _emit_body(nc, tc, pools, drams, parts=("dma", "mm", "epi", "out")):
    consts, fwpool, psum, work = pools
    (ft_dram, wpi_dram, wrhod_dram, wvar_dram, blk_dram,
     opi_dram, omu_dram, ovar_dram) = drams

    BW = 128 + M + GW_RHOD + M
    blk = consts.tile([1, BW], F16, tag="bias_blk", name="bias_blk")
    nc.gpsimd.dma_start(blk[:], blk_dram)

    # Input stream, arrival order = use order: rho weights first, then
    # tile-0 features (tile-0 rho matmuls start ~2.5us in and keep the PE
    # busy-streak alive), tile-1 features, then pi and var weight groups.
    # Each group's epilogue chain hides under the remaining stream; only
    # var's short chain trails it.
    wrhod = fwpool.tile([128, KC, GW_RHOD], F16, tag="wrhod", name="wrhod")
    nc.sync.dma_start(wrhod[:], wrhod_dram)
    ft = fwpool.tile([128, NT, KC, 128], F16, tag="ft", name="ft")
    for t in range(NT):
        nc.sync.dma_start(ft[:, t], ft_dram[:, t])
    wpi = fwpool.tile([128, KC, M], F16, tag="wpi", name="wpi")
    nc.sync.dma_start(wpi[:], wpi_dram)
    wvar = fwpool.tile([128, KC, M], F16, tag="wvar", name="wvar")
    nc.sync.dma_start(wvar[:], wvar_dram)
    if "mm" not in parts:
        return

    # PSUM: one [128, 2, 512] tile (2 banks) per column group; matmul
    # dests are the per-tile halves, epilogue reads stacked strided APs.
    P_pi = psum.tile([128, NT, 512], F32, tag="P_pi", name="P_pi")
    P_rhod = psum.tile([128, NT, 512], F32, tag="P_rhod", name="P_rhod")
    P_var = psum.tile([128, NT, 512], F32, tag="P_var", name="P_var")
    scratch = psum.tile([128, 512], F32, tag="pe_scratch2", name="pe_scratch2")
    fw = consts.tile([1, 128], F16, tag="fill_w", name="fill_w")
    nc.vector.memset(fw[:], 1.0)
    fm = consts.tile([1, 512], F16, tag="fill_m", name="fill_m")
    nc.vector.memset(fm[:], 1.0)

    b_pi = blk[:, 128:128 + M]
    b_rhod = blk[:, 128 + M:128 + M + GW_RHOD]
    b_var = blk[:, 128 + M + GW_RHOD:BW]
    ones = blk[:, 0:128]
    for t in range(NT):
        nc.tensor.matmul(P_rhod[:, t, 0:GW_RHOD], ones, b_rhod,
                         start=True, stop=False)
        nc.tensor.matmul(P_pi[:, t, 0:M], ones, b_pi, start=True, stop=False)
        nc.tensor.matmul(P_var[:, t, 0:M], ones, b_var, start=True, stop=False)
    emit_epi = "epi" in parts
    emit_out = "out" in parts and emit_epi
    if emit_epi:
        # Flat [128, NT*M] output tiles -> one 512B descriptor/partition.
        o_pi = work.tile([128, NT * M], F16, tag="o_pi", name="o_pi")
        o_mu = work.tile([128, NT * M], F16, tag="o_mu", name="o_mu")
        o_var = work.tile([128, NT * M], F16, tag="o_var", name="o_var")

    # Each head's epilogue is EMITTED right after its matmul group: the
    # tile scheduler follows emission order per engine, so this biases it
    # to start each ACT/DVE chain at the earliest legal point instead of
    # batching them after later matmul groups.

    # --- rho group: matmuls per tile as its features land, then r/s/mu ---
    for c in range(KC):
        nc.tensor.matmul(P_rhod[:, 0, 0:GW_RHOD], ft[:, 0, c, :],
                         wrhod[:, c, :], start=False, stop=(c == KC - 1))
    for _ in range(N_FILL_MID):
        nc.tensor.matmul(scratch[:], fw[:], fm[:], start=True, stop=True)
    for c in range(KC):
        nc.tensor.matmul(P_rhod[:, 1, 0:GW_RHOD], ft[:, 1, c, :],
                         wrhod[:, c, :], start=False, stop=(c == KC - 1))
    if emit_epi:
        r = work.tile([128, NT, M], F16, tag="r", name="r")
        nc.scalar.activation(r[:], P_rhod[:, :, 0:M], AF.Tanh, scale=-1.0)
        eneg = work.tile([128, NT, M], F16, tag="eneg", name="eneg")
        nc.scalar.activation(eneg[:], P_rhod[:, :, 0:M], AF.Exp)
        erho = work.tile([128, NT, M], F16, tag="erho", name="erho")
        nc.scalar.activation(erho[:], r[:], AF.Exp)

        s = work.tile([128, NT, M], F16, tag="s", name="s")
        nc.vector.scalar_tensor_tensor(s[:], r[:], 1.0, eneg[:], OP.add,
                                       OP.mult)
        ss = work.tile([128, NT, M], F16, tag="ss", name="ss")
        nc.gpsimd.tensor_mul(ss[:], s[:], s[:])
        q = work.tile([128, NT, M], F16, tag="q", name="q")
        for t in range(NT):
            # q = d1 + s*d2; d-scalars read straight out of PSUM
            nc.vector.tensor_scalar(q[:, t, :], s[:, t, :],
                                    P_rhod[:, t, M + 1:M + 2],
                                    P_rhod[:, t, M:M + 1], OP.mult, OP.add)
        rq = work.tile([128, NT, M], F16, tag="rq", name="rq")
        nc.vector.tensor_mul(rq[:], r[:], q[:])
        for t in range(NT):
            # mu = ss*d3 + rq
            nc.vector.scalar_tensor_tensor(o_mu[:, t * M:(t + 1) * M],
                                           ss[:, t, :],
                                           P_rhod[:, t, M + 2:M + 3],
                                           rq[:, t, :], OP.mult, OP.add)
        if emit_out:
            nc.gpsimd.dma_start(omu_dram, o_mu[:])

    # --- pi group: matmuls, then softmax (stacked exp, DVE reduce) ---
    for c in range(KC):
        for t in range(NT):
            nc.tensor.matmul(P_pi[:, t, 0:M], ft[:, t, c, :], wpi[:, c, :],
                             start=False, stop=(c == KC - 1))
    if emit_epi:
        e_pi = work.tile([128, NT, M], F16, tag="e_pi", name="e_pi")
        nc.scalar.activation(e_pi[:], P_pi[:, :, 0:M], AF.Exp)
        ssum = work.tile([128, NT], F32, tag="ssum", name="ssum")
        nc.vector.tensor_reduce(ssum[:], e_pi[:], mybir.AxisListType.X,
                                OP.add)
        rsum = work.tile([128, NT], F32, tag="rsum", name="rsum")
        nc.vector.reciprocal(rsum[:], ssum[:])
        for t in range(NT):
            nc.vector.tensor_scalar_mul(o_pi[:, t * M:(t + 1) * M],
                                        e_pi[:, t, :], rsum[:, t:t + 1])
        if emit_out:
            nc.sync.dma_start(opi_dram, o_pi[:])

    # --- var group (tail): matmuls, then var = -(erho-1)*var0 + tau ---
    for c in range(KC):
        for t in range(NT):
            nc.tensor.matmul(P_var[:, t, 0:M], ft[:, t, c, :], wvar[:, c, :],
                             start=False, stop=(c == KC - 1))
    if emit_epi:
        ev = work.tile([128, NT, M], F16, tag="ev", name="ev")
        nc.scalar.activation(ev[:], P_var[:, :, 0:M], AF.Exp)
        t1 = work.tile([128, NT, M], F16, tag="t1", name="t1")
        nc.vector.scalar_tensor_tensor(t1[:], erho[:], 1.0, ev[:],
                                       OP.subtract, OP.mult)
        for t in range(NT):
            nc.vector.tensor_scalar(o_var[:, t * M:(t + 1) * M], t1[:, t, :],
                                    -1.0, TAU_INV, OP.mult, OP.add)
        if emit_out:
            nc.scalar.dma_start(ovar_dram, o_var[:])


def _build_pools(tc, ctx):
    consts = ctx.enter_context(tc.tile_pool(name="consts", bufs=1))
    fwpool = ctx.enter_context(tc.tile_pool(name="fw", bufs=1))
    psum = ctx.enter_context(tc.tile_pool(name="psum", bufs=1, space="PSUM"))
    work = ctx.enter_context(tc.tile_pool(name="work", bufs=1))
    return consts, fwpool, psum, work


def _build_nc():
    nc = bacc.Bacc("TRN2", target_bir_lowering=False, debug=False)
    drams = _declare_io(nc)
    with tile.TileContext(nc) as tc, ExitStack() as ctx:
        consts, fwpool, psum, work = _build_pools(tc, ctx)
        scratch = psum.tile([128, 512], F32, tag="pe_scratch",
                            name="pe_scratch")
        _warmup_act(nc, consts)
        _warmup_pe(nc, consts, scratch)
        _emit_body(nc, tc, (consts, fwpool, psum, work), drams)
    nc.compile()
    return nc


def build_loop_nc(reps, parts=("dma", "mm", "epi", "out"), fillers=True):
    """Timing variant: run the body `reps` times inside one NEFF (used only
    by the local test harness; the default full-barrier back-edge keeps
    iterations serialized so per-iter span ~ single-shot kernel time)."""
    nc = bacc.Bacc("TRN2", target_bir_lowering=False, debug=False)
    drams = _declare_io(nc)
    with tile.TileContext(nc) as tc, ExitStack() as ctx:
        consts, fwpool, psum, work = _build_pools(tc, ctx)
        scratch = psum.tile([128, 512], F32, tag="pe_scratch",
                            name="pe_scratch")
        _warmup_act(nc, consts)
        with tc.For_i(0, reps, 1):
            if fillers:
                _warmup_pe(nc, consts, scratch)
            _emit_body(nc, tc, (consts, fwpool, psum, work), drams,
                       parts=parts)
    nc.compile()
    return nc


_CACHE = {}


def _get_nc():
    if "nc" not in _CACHE:
        _CACHE["nc"] = _build_nc()
    return _CACHE["nc"]


def _host_prep(inputs):
    f32 = np.float32
    feature = np.ascontiguousarray(inputs["feature"], dtype=f32)
    muW = np.asarray(inputs["muW"], dtype=f32)
    W = np.asarray(inputs["W"], dtype=f32)
    Z = np.asarray(inputs["Z"], dtype=f32)
    logvarW = np.asarray(inputs["logvarW"], dtype=f32)
    logvarZ = np.asarray(inputs["logvarZ"], dtype=f32)

    wstd = np.sqrt(np.exp(logvarW)).astype(f32)
    zstd = np.sqrt(np.exp(logvarZ)).astype(f32)
    a = ((zstd / wstd).astype(f32) * (W - muW)).astype(f32)

    # Column groups; rho weights negated so psum = -u and exp(psum) = e^-u.
    wpi = np.asarray(inputs["h2pi_w"], dtype=f32).T          # [F, M]
    wrho = -np.asarray(inputs["h2rho_w"], dtype=f32).T       # [F, M]
    wrho[:, 0] = 0.0                                         # folded clamp
    wvar = np.asarray(inputs["h2var_w"], dtype=f32).T        # [F, M]
    wrhod = np.concatenate(
        [wrho, np.stack([muW, a, Z, np.zeros_like(muW)], axis=1)], axis=1)

    b_pi = np.asarray(inputs["h2pi_b"], dtype=f32)
    b_rho = -np.asarray(inputs["h2rho_b"], dtype=f32)
    b_rho[0] = -U0                                           # folded clamp
    b_var = np.asarray(inputs["h2var_b"], dtype=f32)
    blk = np.concatenate(
        [np.ones(128, dtype=f32), b_pi, b_rho, np.zeros(4, dtype=f32),
         b_var]).reshape(1, -1).astype(MM_NP)

    # [F, gw] -> [128(p), KC, gw]
    def wfmt(w):
        return np.ascontiguousarray(
            w.reshape(KC, 128, w.shape[1]).transpose(1, 0, 2), dtype=MM_NP)

    wpi_h, wrhod_h, wvar_h = wfmt(wpi), wfmt(wrhod), wfmt(wvar)

    in_maps = []
    for cr in range(NCORES):
        shard = feature[cr * BC:(cr + 1) * BC]               # [BC, F]
        # ft[p, t, c, b] = shard[t*128+b, c*128+p]
        ft = np.ascontiguousarray(
            shard.reshape(NT, 128, KC, 128).transpose(3, 0, 2, 1),
            dtype=MM_NP)
        in_maps.append({"ft": ft, "wpi": wpi_h, "wrhod": wrhod_h,
                        "wvar": wvar_h, "bias_blk": blk})
    return in_maps


def _postprocess(res, cores):
    outs = []
    for name in ("out_pi", "out_mu", "out_var"):
        # [128, NT*M] partition-major -> [BC, M] rows per core
        full = np.concatenate(
            [np.asarray(res.results[c][name], dtype=np.float32)
             .reshape(128, NT, M).transpose(1, 0, 2).reshape(BC, M)
             for c in cores], axis=0)
        outs.append(np.ascontiguousarray(full))
    return tuple(outs)


def kernel(**inputs):
    nc = _get_nc()
    in_maps = _host_prep(inputs)
    res = run_bass_kernel_spmd(nc, in_maps, list(range(NCORES)))
    return _postprocess(res, list(range(NCORES)))
